# revision 1
# baseline (speedup 1.0000x reference)
"""Bidirectional LSTM on 8 trn2 NeuronCores.

Sharding: 2 directions x 4-way batch split (B_local=8 per core). Every core
runs the IDENTICAL forward-scan program; backward cores receive
time-reversed x and their outputs are re-reversed on the host. The scan is
fully core-local: per-step cross-core exchange (~us DMA/collective latency
x 512 steps) would dominate, so batch x direction is the only sharding that
keeps the recurrence off the wire.

Per-core plan (B=8, T=512, I=256, H=512, G=4H=2048):
  1. Host pre-transposes/casts weights and x to fp16 (lhsT / moving-operand
     layouts) -- no on-chip transpose phase, minimal transfer bytes.
  2. Precompute xp = x @ W_ih.T + b for all T into an SBUF-resident fp16
     buffer (gates.T layout).
  3. 512-step scan: per gate-group, 16 matmuls [128x128]x[128x8] accumulate
     gates.T in a dedicated PSUM bank (4 gates x double-buffered = all 8
     banks); DVE adds xp; ACT sigmoid/tanh; DVE cell update; h stays fp16
     as the next step's moving operand. Gate order g,i,f,o keeps tanh(c)
     off the critical tail. This is the PE weight-ingestion floor: W_hh
     must stream through the array every step (64 tiles; fp16 + FWL is the
     fastest legal stationary load).
  4. Output written windowed in hardware-native layout (fp16), unscrambled
     and upcast on the host.

The compiled PJRT executable is cached at module level: repeat kernel()
calls only transfer fresh inputs and execute (~1.4 s vs ~40+ s).
"""

import numpy as np

B_FULL, T, I, H = 32, 512, 256, 512
G = 4 * H
N_CORES = 8
B = B_FULL // 4          # per-core batch
KH = H // 128            # 4 k-chunks for W_hh
KI = I // 128            # 2 k-chunks for W_ih
M = G // 128             # 16 m-chunks (4 per gate)
WIN = 16                 # scan steps per output DMA window
T_SCAN = T

_BUILT = {}


def _install_tile_patch():
    """This container's walrus accepts only ONE sync-wait per CTRL-class
    instruction (Drain/NoOp). Tile's kernel-tail drain aggregates one wait
    per semaphore lane onto a single Drain -> split them one per drain."""
    import bass_rust
    import concourse.tile as tile

    if getattr(tile.TileContext, "_drain_split_patched", False):
        return

    def _patched_dab(self, tick_clock, wait_clock):
        from concourse.tile import ScopedClock

        nc = self.nc
        drain_inst = nc.sync.drain()
        wait_clock.add_sem_waits(
            drain_inst.ins, ScopedClock({None: tick_clock.global_clock})
        )
        si = drain_inst.ins.sync_info
        waits = list(si.on_wait) if si is not None else []
        if len(waits) > 1:
            si.on_wait = waits[:1]
            for w in waits[1:]:
                d2 = nc.sync.drain()
                si2 = d2.ins.sync_info
                if si2 is None:
                    d2.ins.sync_info = bass_rust.SyncInfo(on_wait=[w], on_update=[])
                else:
                    si2.on_wait = list(si2.on_wait) + [w]
        nc.all_engine_barrier()
        assert self.sems is not None
        popped = nc._tile_sem_poison_stack.pop()
        assert popped is self._sem_poison
        nc.clear_and_free_semaphores(list(self.sems.allocated().values()))
        nc.all_engine_barrier()

    tile.TileContext._drain_and_barrier = _patched_dab
    tile.TileContext._drain_split_patched = True

    # This walrus build accepts at most ONE sync-wait per instruction (any
    # opcode). Split every multi-wait instruction at BIR-JSON level into
    # single-wait NoOps followed by the real instruction with one wait.
    import json
    import concourse.bass as bass

    if getattr(bass.Bass, "_json_wait_split_patched", False):
        return
    _orig_tjb = bass.Bass.to_json_bytes

    def _split_json(self):
        raw = _orig_tjb(self)
        m = json.loads(raw)
        ctr = 0
        changed = False
        for fn in m.get("functions", []):
            for bb in fn.get("blocks", []):
                out = []
                for inst in bb.get("instructions", []):
                    si = inst.get("sync_info")
                    waits = (si or {}).get("on_wait") or []
                    if len(waits) > 1:
                        changed = True
                        for w in waits[:-1]:
                            ctr += 1
                            nop = {
                                "engine": inst["engine"],
                                "ins": [],
                                "outs": [],
                                "name": f"WSPLIT-{ctr}",
                                "opcode": "NoOp",
                                "sync_info": {"on_update": [], "on_wait": [w]},
                            }
                            if "debug" in inst:
                                nop["debug"] = inst["debug"]
                            out.append(nop)
                        si["on_wait"] = [waits[-1]]
                    out.append(inst)
                bb["instructions"] = out
        if not changed:
            return raw
        return json.dumps(m).encode()

    bass.Bass.to_json_bytes = _split_json
    bass.Bass._json_wait_split_patched = True


def _build(t_scan):
    import concourse.bass as bass
    import concourse.tile as tile
    from concourse import mybir
    from contextlib import ExitStack

    _install_tile_patch()
    f32 = mybir.dt.float32
    f16 = mybir.dt.float16

    nc = bass.Bass()
    # Host pre-transposes/casts: xT [I, B*T] f16, whhT [H, G] f16,
    # wihT [I, G] f16, b_sb [128, M] f32.
    xt_d = nc.dram_tensor("xT", [I, B * T], f16, kind="ExternalInput")
    wiht_d = nc.dram_tensor("wihT", [I, G], f16, kind="ExternalInput")
    whht_d = nc.dram_tensor("whhT", [H, G], f16, kind="ExternalInput")
    bsb_d = nc.dram_tensor("bsb", [128, M], f32, kind="ExternalInput")
    n_win = (t_scan + WIN - 1) // WIN
    out_d = nc.dram_tensor("out_raw", [n_win, 128, WIN * 4 * B], f16,
                           kind="ExternalOutput")

    TB = B * T  # 4096 flattened (b, t) columns, b-major

    with tile.TileContext(nc) as tc, ExitStack() as ctx:
        sig = mybir.ActivationFunctionType.Sigmoid
        tanh = mybir.ActivationFunctionType.Tanh

        wpool = ctx.enter_context(tc.tile_pool(name="w", bufs=1))
        whhT = wpool.tile([128, KH * M * 128], f16)   # tile (k,m) at (k*M+m)*128
        wihT = wpool.tile([128, KI * M * 128], f16)
        xT = wpool.tile([128, KI * TB], f16)          # k-chunk ki at ki*TB
        xp = wpool.tile([128, M * TB], f16)           # chunk m at m*TB, col b*T+t
        b_sb = wpool.tile([128, M], f32)
        nc.gpsimd.dma_start(b_sb[:], bsb_d[:])
        for k in range(KH):
            nc.gpsimd.dma_start(whhT[:, k * G:(k + 1) * G],
                                whht_d[k * 128:(k + 1) * 128, :])
        for k in range(KI):
            nc.gpsimd.dma_start(wihT[:, k * G:(k + 1) * G],
                                wiht_d[k * 128:(k + 1) * 128, :])
            nc.gpsimd.dma_start(xT[:, k * TB:(k + 1) * TB],
                                xt_d[k * 128:(k + 1) * 128, :])

        # ---- phase C: xp = x @ W_ih.T + b, fp16, gates.T layout ----
        NXP = 512
        with tc.tile_pool(name="xppsum", bufs=4, space="PSUM") as xpp:
            for m in range(M):
                for n in range(TB // NXP):
                    ps = xpp.tile([128, NXP], f32, tag="xps")
                    for k in range(KI):
                        nc.tensor.matmul(
                            ps[:],
                            wihT[:, (k * M + m) * 128:(k * M + m + 1) * 128],
                            xT[:, k * TB + n * NXP:k * TB + (n + 1) * NXP],
                            start=(k == 0), stop=(k == KI - 1),
                        )
                    dst = xp[:, m * TB + n * NXP:m * TB + (n + 1) * NXP]
                    if n % 2 == 0:
                        nc.vector.tensor_scalar_add(dst, ps[:], b_sb[:, m:m + 1])
                    else:
                        nc.scalar.add(dst, ps[:], b_sb[:, m:m + 1])

        # ---- phase D: the scan ----
        # col layout of h/c/gate tiles: 8k + b  (k = H 128-chunk, b = batch)
        xp4 = xp.rearrange("p (m b t) -> p m b t", m=M, b=B)
        with tc.tile_pool(name="gpsum", bufs=4, space="PSUM") as gp, \
             tc.tile_pool(name="acts", bufs=2) as apool, \
             tc.tile_pool(name="state", bufs=2) as stp, \
             tc.tile_pool(name="outb", bufs=2) as obp:
            h_prev = stp.tile([128, KH * B], f16, tag="h")
            c_prev = stp.tile([128, KH * B], f32, tag="c")
            nc.vector.memset(h_prev[:], 0.0)
            nc.vector.memset(c_prev[:], 0.0)

            # Gates packed pairwise into PSUM tiles so the pool can run
            # bufs=4 (2 tags x 4 bufs x 1 bank = 8 banks): WAR distance 4
            # steps keeps slot-reuse waits pre-satisfied (no PE issue
            # stalls). Pair (g,o) in tile A cols [0:32|32:64], (i,f) in
            # tile B. Emission order g,i,f,o keeps tanh(c) off the tail.
            GB = KH * B  # 32 cols per gate
            ob = None
            for t in range(t_scan):
                s = t % WIN
                if s == 0:
                    ob = obp.tile([128, WIN * GB], f32, tag="ob")
                ps_a = gp.tile([128, 2 * GB], f32, tag="psA")  # g | o
                ps_b = gp.tile([128, 2 * GB], f32, tag="psB")  # i | f
                place = {2: (ps_a, 0), 0: (ps_b, 0), 1: (ps_b, GB),
                         3: (ps_a, GB)}
                for g in (2, 0, 1, 3):  # g, i, f, o
                    ps, base = place[g]
                    for mi in range(KH):
                        m = 4 * g + mi
                        for k in range(KH):
                            nc.tensor.matmul(
                                ps[:, base + 8 * mi:base + 8 * mi + 8],
                                whhT[:, (k * M + m) * 128:(k * M + m + 1) * 128],
                                h_prev[:, 8 * k:8 * k + 8],
                                start=(k == 0), stop=(k == KH - 1),
                            )
                # g: add xp + tanh as soon as its 16 MMs are done
                sg_g = apool.tile([128, GB], f32, tag="sgg")
                nc.vector.tensor_add(
                    sg_g.rearrange("p (m b) -> p m b", m=KH),
                    ps_a[:, 0:GB].rearrange("p (m b) -> p m b", m=KH),
                    xp4[:, 8:12, :, t],
                )
                ac_g = apool.tile([128, GB], f32, tag="acg")
                nc.scalar.activation(ac_g[:], sg_g[:], tanh)
                # i,f: fused add + single sigmoid over both (xp chunks 0-7)
                sg_if = apool.tile([128, 2 * GB], f32, tag="sgif")
                nc.vector.tensor_add(
                    sg_if.rearrange("p (m b) -> p m b", m=2 * KH),
                    ps_b.rearrange("p (m b) -> p m b", m=2 * KH),
                    xp4[:, 0:8, :, t],
                )
                ac_if = apool.tile([128, 2 * GB], f32, tag="acif")
                nc.scalar.activation(ac_if[:], sg_if[:], sig)
                # o last (tail: add + sigmoid + h-mul only)
                sg_o = apool.tile([128, GB], f32, tag="sgo")
                nc.vector.tensor_add(
                    sg_o.rearrange("p (m b) -> p m b", m=KH),
                    ps_a[:, GB:2 * GB].rearrange("p (m b) -> p m b", m=KH),
                    xp4[:, 12:16, :, t],
                )
                ac_o = apool.tile([128, GB], f32, tag="aco")
                nc.scalar.activation(ac_o[:], sg_o[:], sig)
                ig = apool.tile([128, GB], f32, tag="ig")
                nc.vector.tensor_mul(ig[:], ac_if[:, 0:GB], ac_g[:])
                fc = apool.tile([128, GB], f32, tag="fc")
                nc.vector.tensor_mul(fc[:], ac_if[:, GB:2 * GB], c_prev[:])
                c_new = stp.tile([128, GB], f32, tag="c")
                nc.vector.tensor_add(c_new[:], ig[:], fc[:])
                th = apool.tile([128, GB], f32, tag="th")
                nc.scalar.activation(th[:], c_new[:], tanh)
                h_new = stp.tile([128, GB], f16, tag="h")
                nc.vector.tensor_mul(h_new[:], ac_o[:], th[:])
                nc.vector.tensor_mul(ob[:, 32 * s:32 * s + 32], ac_o[:], th[:])
                h_prev, c_prev = h_new, c_new
                if s == WIN - 1 or t == t_scan - 1:
                    nc.gpsimd.dma_start(out_d[t // WIN], ob[:])

    return nc


def _get_nc(t_scan):
    key = t_scan
    if key not in _BUILT:
        _BUILT[key] = _build(t_scan)
    return _BUILT[key]


_RUNNERS = {}


def _make_runner(t_scan):
    """Compile once, return a callable in_maps -> list[dict] that only
    executes (PJRT executable cached across kernel() calls). Donated output
    buffers are created on-device (jnp.zeros) so they are never shipped
    from the host."""
    import jax
    import jax.numpy as jnp
    import numpy as np
    from jax.sharding import Mesh, PartitionSpec
    from jax.experimental.shard_map import shard_map
    from concourse import bass2jax, mybir
    from concourse.bass2jax import _bass_exec_p, install_neuronx_cc_hook

    install_neuronx_cc_hook()
    nc = _get_nc(t_scan)
    assert nc.dbg_addr is None
    n_cores = N_CORES
    partition_name = (nc.partition_id_tensor.name
                      if nc.partition_id_tensor else None)
    in_names, out_names, out_avals, zero_shapes = [], [], [], []
    for alloc in nc.m.functions[0].allocations:
        if not isinstance(alloc, mybir.MemoryLocationSet):
            continue
        name = alloc.memorylocations[0].name
        if alloc.kind == "ExternalInput":
            if name != partition_name:
                in_names.append(name)
        elif alloc.kind == "ExternalOutput":
            shape = tuple(alloc.tensor_shape)
            npdt = mybir.dt.np(alloc.dtype)
            out_avals.append(jax.core.ShapedArray(shape, npdt))
            out_names.append(name)
            zero_shapes.append((shape, npdt))
    n_params = len(in_names)
    n_outs = len(out_names)
    all_in = in_names + out_names
    if partition_name is not None:
        all_in = all_in + [partition_name]

    def _body(*args):
        operands = list(args)
        if partition_name is not None:
            operands.append(bass2jax.partition_id_tensor())
        outs = _bass_exec_p.bind(
            *operands,
            out_avals=tuple(out_avals),
            in_names=tuple(all_in),
            out_names=tuple(out_names),
            lowering_input_output_aliases=(),
            sim_require_finite=True,
            sim_require_nnan=True,
            nc=nc,
        )
        return tuple(outs)

    devices = jax.devices()[:n_cores]
    mesh = Mesh(np.asarray(devices), ("core",))
    donate = tuple(range(n_params, n_params + n_outs))
    sharded = jax.jit(
        shard_map(_body, mesh=mesh,
                  in_specs=(PartitionSpec("core"),) * (n_params + n_outs),
                  out_specs=(PartitionSpec("core"),) * n_outs,
                  check_rep=False),
        donate_argnums=donate, keep_unused=True,
    )

    def run(in_maps):
        concat_in = [
            np.concatenate([np.asarray(m[name]) for m in in_maps], axis=0)
            for name in in_names
        ]
        concat_zeros = [
            jnp.zeros((n_cores * s[0], *s[1:]), dt) for s, dt in zero_shapes
        ]
        out_arrs = sharded(*concat_in, *concat_zeros)
        return [
            {name: np.asarray(out_arrs[i]).reshape(
                n_cores, *out_avals[i].shape)[c]
             for i, name in enumerate(out_names)}
            for c in range(n_cores)
        ]

    return run


def _run_spmd(t_scan, in_maps):
    if t_scan not in _RUNNERS:
        try:
            _RUNNERS[t_scan] = _make_runner(t_scan)
        except Exception:
            _RUNNERS[t_scan] = None
    runner = _RUNNERS[t_scan]
    if runner is not None:
        return runner(in_maps)
    from concourse.bass_utils import run_bass_kernel_spmd
    res = run_bass_kernel_spmd(_get_nc(t_scan), in_maps, list(range(N_CORES)))
    return res.results


def kernel(x, W_ih_f, W_hh_f, b_f, W_ih_b, W_hh_b, b_b, _t_scan=T_SCAN):
    x = np.asarray(x, dtype=np.float32)
    params = {}
    for d, (wih, whh, bb) in enumerate(
            [(W_ih_f, W_hh_f, b_f), (W_ih_b, W_hh_b, b_b)]):
        wih = np.asarray(wih, np.float32)
        whh = np.asarray(whh, np.float32)
        bb = np.asarray(bb, np.float32)
        params[d] = (
            np.ascontiguousarray(wih.T).astype(np.float16),     # [I, G]
            np.ascontiguousarray(whh.T).astype(np.float16),     # [H, G]
            np.ascontiguousarray(bb.reshape(M, 128).T),         # [128, M]
        )
    in_maps = []
    for c in range(N_CORES):
        d = c // 4          # 0 = forward, 1 = backward
        bs = (c % 4) * B
        xs = x[bs:bs + B]
        if d == 1:
            xs = xs[:, ::-1]
        xt = np.ascontiguousarray(
            xs.reshape(B * T, I).T).astype(np.float16)          # [I, B*T]
        wiht, whht, bsb = params[d]
        in_maps.append({
            "xT": xt, "wihT": wiht, "whhT": whht, "bsb": bsb,
        })

    results = _run_spmd(_t_scan, in_maps)

    n_win = (_t_scan + WIN - 1) // WIN
    t_out = n_win * WIN
    halves = []
    for d in range(2):
        parts = []
        for c4 in range(4):
            raw = np.asarray(results[d * 4 + c4]["out_raw"])
            # raw[w, p, 32s + 8k + b] = h[b, 16w+s, 128k+p]
            h = raw.reshape(n_win, 128, WIN, KH, B)
            h = np.ascontiguousarray(h.transpose(4, 0, 2, 3, 1))
            h = h.reshape(B, t_out, H)[:, :_t_scan]
            parts.append(h)
        hcat = np.concatenate(parts, axis=0)
        if d == 1:
            hcat = hcat[:, ::-1]
        halves.append(hcat)
    return np.concatenate(halves, axis=2).astype(np.float32)



# revision 2
# speedup vs baseline: 29.8028x; 29.8028x over previous
"""Bidirectional LSTM on trn2 NeuronCores.

Sharding: 2 cores, one per direction, full batch B=32 per core. The
backward core receives time-reversed x and its output is re-reversed on
the host. The scan is fully core-local (the recurrence never crosses the
wire), and using 2 cores instead of 8 minimizes total device-seconds:
the scan cost is dominated by W_hh stationary-weight ingestion into the
PE array (64 LDWEIGHTS x 128x128 fp16 per step), which is independent of
the per-core batch size, so batch-splitting across more cores multiplies
device time without reducing latency.

Per-core plan (B=32, T=512, I=256, H=512, G=4H=2048):
  1. Host pre-transposes/casts weights and x to fp16 (lhsT / moving
     layouts, t-major x). Gate blocks permuted to i,f,o,g so sigmoid
     covers one contiguous 96-col span per chunk.
  2. Phase C: xp = x @ W_ih.T + b for all T, written to a DRAM buffer in
     16-step window layout (doesn't fit SBUF at B=32), N=512 moving
     columns per matmul so LDWEIGHTS is fully amortized.
  3. 512-step scan, chunk-pipelined: gates are computed per H-chunk k
     (16 matmuls -> PSUM tile [128, 4x32]), and each chunk's
     DVE/ACT tail (add xp, sigmoid/tanh, cell update) runs while the PE
     streams the next chunk's weights. h chunks are written straight
     into the fp16 output window tile, which doubles as next step's
     moving operand, so the serial tail at a step boundary is one chunk
     deep instead of a full step.
  4. xp windows stream DRAM->SBUF double-buffered; output windows
     (16 steps) DMA out as they complete; host unscrambles + upcasts.

The compiled PJRT executable is cached at module level: repeat kernel()
calls only transfer fresh inputs and execute.
"""

import numpy as np

B_FULL, T, I, H = 32, 512, 256, 512
G = 4 * H
N_CORES = 2
B = B_FULL                # per-core batch (one direction per core)
KH = H // 128             # 4 contraction chunks for W_hh
KI = I // 128             # 2 contraction chunks for W_ih
M = G // 128              # 16 gate-row chunks (4 per gate)
WIN = 16                  # scan steps per xp/output window
TB = T * B                # 16384 moving columns, t-major
WCOL = M * WIN * B        # 8192 xp columns per window
T_SCAN = T

_BUILT = {}


def _install_tile_patch():
    """This container's walrus accepts only ONE sync-wait per CTRL-class
    instruction (Drain/NoOp). Tile's kernel-tail drain aggregates one wait
    per semaphore lane onto a single Drain -> split them one per drain."""
    import bass_rust
    import concourse.tile as tile

    if getattr(tile.TileContext, "_drain_split_patched", False):
        return

    def _patched_dab(self, tick_clock, wait_clock):
        from concourse.tile import ScopedClock

        nc = self.nc
        drain_inst = nc.sync.drain()
        wait_clock.add_sem_waits(
            drain_inst.ins, ScopedClock({None: tick_clock.global_clock})
        )
        si = drain_inst.ins.sync_info
        waits = list(si.on_wait) if si is not None else []
        if len(waits) > 1:
            si.on_wait = waits[:1]
            for w in waits[1:]:
                d2 = nc.sync.drain()
                si2 = d2.ins.sync_info
                if si2 is None:
                    d2.ins.sync_info = bass_rust.SyncInfo(on_wait=[w], on_update=[])
                else:
                    si2.on_wait = list(si2.on_wait) + [w]
        nc.all_engine_barrier()
        assert self.sems is not None
        popped = nc._tile_sem_poison_stack.pop()
        assert popped is self._sem_poison
        nc.clear_and_free_semaphores(list(self.sems.allocated().values()))
        nc.all_engine_barrier()

    tile.TileContext._drain_and_barrier = _patched_dab
    tile.TileContext._drain_split_patched = True

    # This walrus build accepts at most ONE sync-wait per instruction (any
    # opcode). Split every multi-wait instruction at BIR-JSON level into
    # single-wait NoOps followed by the real instruction with one wait.
    import json
    import concourse.bass as bass

    if getattr(bass.Bass, "_json_wait_split_patched", False):
        return
    _orig_tjb = bass.Bass.to_json_bytes

    def _split_json(self):
        raw = _orig_tjb(self)
        m = json.loads(raw)
        ctr = 0
        changed = False
        for fn in m.get("functions", []):
            for bb in fn.get("blocks", []):
                out = []
                for inst in bb.get("instructions", []):
                    si = inst.get("sync_info")
                    waits = (si or {}).get("on_wait") or []
                    if len(waits) > 1:
                        changed = True
                        for w in waits[:-1]:
                            ctr += 1
                            nop = {
                                "engine": inst["engine"],
                                "ins": [],
                                "outs": [],
                                "name": f"WSPLIT-{ctr}",
                                "opcode": "NoOp",
                                "sync_info": {"on_update": [], "on_wait": [w]},
                            }
                            if "debug" in inst:
                                nop["debug"] = inst["debug"]
                            out.append(nop)
                        si["on_wait"] = [waits[-1]]
                    out.append(inst)
                bb["instructions"] = out
        if not changed:
            return raw
        return json.dumps(m).encode()

    bass.Bass.to_json_bytes = _split_json
    bass.Bass._json_wait_split_patched = True


def _build(t_scan):
    import concourse.bass as bass
    import concourse.tile as tile
    from concourse import mybir
    from contextlib import ExitStack

    _install_tile_patch()
    f32 = mybir.dt.float32
    f16 = mybir.dt.float16

    assert t_scan % WIN == 0
    n_win = t_scan // WIN

    nc = bass.Bass()
    # Host layouts: xT [I, T*B] f16 t-major (col t*B + b), wihT [I, G] f16,
    # whhT [H, G] f16 (G rows permuted to gate order i,f,o,g), bsb [128, M].
    xt_d = nc.dram_tensor("xT", [I, TB], f16, kind="ExternalInput")
    wiht_d = nc.dram_tensor("wihT", [I, G], f16, kind="ExternalInput")
    whht_d = nc.dram_tensor("whhT", [H, G], f16, kind="ExternalInput")
    bsb_d = nc.dram_tensor("bsb", [128, M], f32, kind="ExternalInput")
    # out[w, p, s*128 + k*32 + b] = h[b, 16w+s, 128k+p]
    out_d = nc.dram_tensor("out_raw", [n_win, 128, WIN * KH * B], f16,
                           kind="ExternalOutput")

    with tile.TileContext(nc) as tc, ExitStack() as ctx:
        sig = mybir.ActivationFunctionType.Sigmoid
        tanh = mybir.ActivationFunctionType.Tanh

        wpool = ctx.enter_context(tc.tile_pool(name="w", bufs=1))
        dpool = ctx.enter_context(tc.tile_pool(name="d", bufs=1, space="DRAM"))
        whhT = wpool.tile([128, KH * G], f16)    # tile (kk,m) at (kk*M+m)*128
        wihT = wpool.tile([128, KI * G], f16)
        xT = wpool.tile([128, KI * TB], f16)     # chunk ki at ki*TB
        b_sb = wpool.tile([128, M], f32)
        # xp DRAM buffer, window layout: col w*WCOL + m*(WIN*B) + s*B + b
        xp_dram = dpool.tile([128, n_win * WCOL], f16)
        nc.gpsimd.dma_start(b_sb[:], bsb_d[:])
        for k in range(KH):
            nc.gpsimd.dma_start(whhT[:, k * G:(k + 1) * G],
                                whht_d[k * 128:(k + 1) * 128, :])
        for k in range(KI):
            nc.gpsimd.dma_start(wihT[:, k * G:(k + 1) * G],
                                wiht_d[k * 128:(k + 1) * 128, :])
            nc.gpsimd.dma_start(xT[:, k * TB:(k + 1) * TB],
                                xt_d[k * 128:(k + 1) * 128, :])

        # ---- phase C: xp = x @ W_ih.T + b -> DRAM, fp16 ----
        NXP = WIN * B  # 512 moving columns = one window of one m-chunk
        with tc.tile_pool(name="xpps", bufs=4, space="PSUM") as xpp, \
             tc.tile_pool(name="xpsb", bufs=4) as xsb:
            for w in range(n_win):
                for m in range(M):
                    ps = xpp.tile([128, NXP], f32, tag="xps")
                    for k in range(KI):
                        nc.tensor.matmul(
                            ps[:],
                            wihT[:, (k * M + m) * 128:(k * M + m + 1) * 128],
                            xT[:, k * TB + w * NXP:k * TB + (w + 1) * NXP],
                            start=(k == 0), stop=(k == KI - 1),
                        )
                    sb = xsb.tile([128, NXP], f16, tag="xsb")
                    if m % 2 == 0:
                        nc.vector.tensor_scalar_add(sb[:], ps[:], b_sb[:, m:m + 1])
                    else:
                        nc.scalar.add(sb[:], ps[:], b_sb[:, m:m + 1])
                    nc.gpsimd.dma_start(
                        xp_dram[:, w * WCOL + m * NXP:w * WCOL + (m + 1) * NXP],
                        sb[:])

        # ---- phase D: the scan ----
        # gate m-chunk = g*4 + k (g in i,f,o,g order; k = H 128-chunk)
        # h/c col layout: k*32 + b
        with tc.tile_pool(name="gp", bufs=8, space="PSUM") as gp, \
             tc.tile_pool(name="xpw", bufs=2) as xpool, \
             tc.tile_pool(name="acts", bufs=4) as ap, \
             tc.tile_pool(name="state", bufs=2) as stp, \
             tc.tile_pool(name="outb", bufs=2) as obp, \
             tc.tile_pool(name="init", bufs=1) as ip:
            h0 = ip.tile([128, KH * B], f16)
            c0 = ip.tile([128, KH * B], f32)
            nc.vector.memset(h0[:], 0.0)
            nc.vector.memset(c0[:], 0.0)

            def load_window(w):
                tl = xpool.tile([128, WCOL], f16, tag="xp")
                nc.gpsimd.dma_start(tl[:], xp_dram[:, w * WCOL:(w + 1) * WCOL])
                return tl

            xpw_cur = load_window(0)
            xpw_next = None
            h_src = h0
            c_prev = c0
            ob = None
            for t in range(t_scan):
                w, s = divmod(t, WIN)
                if s == 0:
                    if w > 0:
                        xpw_cur = xpw_next
                    if w + 1 < n_win:
                        xpw_next = load_window(w + 1)
                    ob = obp.tile([128, WIN * KH * B], f16, tag="ob")
                # xp view: [p, g, k, s, b]
                xp5 = xpw_cur.rearrange("p (g k s b) -> p g k s b",
                                        g=4, k=KH, s=WIN)
                c_t = stp.tile([128, KH * B], f32, tag="c")
                for k in range(KH):
                    ps = gp.tile([128, 4 * B], f32, tag="ps")  # i|f|o|g
                    for g in range(4):
                        m = g * KH + k
                        for kk in range(KH):
                            nc.tensor.matmul(
                                ps[:, g * B:(g + 1) * B],
                                whhT[:, (kk * M + m) * 128:(kk * M + m + 1) * 128],
                                h_src[:, kk * B:(kk + 1) * B],
                                start=(kk == 0), stop=(kk == KH - 1),
                            )
                    # tail for chunk k
                    sg = ap.tile([128, 4 * B], f32, tag="sg")
                    nc.vector.tensor_add(
                        sg.rearrange("p (g b) -> p g b", g=4),
                        ps.rearrange("p (g b) -> p g b", g=4),
                        xp5[:, :, k, s, :],
                    )
                    af = ap.tile([128, 3 * B], f32, tag="af")   # sig(i|f|o)
                    nc.scalar.activation(af[:], sg[:, 0:3 * B], sig)
                    ag = ap.tile([128, B], f32, tag="ag")       # tanh(g)
                    nc.scalar.activation(ag[:], sg[:, 3 * B:4 * B], tanh)
                    ig = ap.tile([128, B], f32, tag="ig")
                    nc.vector.tensor_mul(ig[:], af[:, 0:B], ag[:])
                    fc = ap.tile([128, B], f32, tag="fc")
                    nc.vector.tensor_mul(fc[:], af[:, B:2 * B],
                                         c_prev[:, k * B:(k + 1) * B])
                    nc.vector.tensor_add(c_t[:, k * B:(k + 1) * B],
                                         ig[:], fc[:])
                    th = ap.tile([128, B], f32, tag="th")
                    nc.scalar.activation(th[:], c_t[:, k * B:(k + 1) * B], tanh)
                    nc.vector.tensor_mul(
                        ob[:, s * KH * B + k * B:s * KH * B + (k + 1) * B],
                        af[:, 2 * B:3 * B], th[:])
                h_src = ob[:, s * KH * B:(s + 1) * KH * B]
                c_prev = c_t
                if s == WIN - 1:
                    nc.gpsimd.dma_start(out_d[w], ob[:])

    return nc


def _get_nc(t_scan):
    key = t_scan
    if key not in _BUILT:
        _BUILT[key] = _build(t_scan)
    return _BUILT[key]


_RUNNERS = {}


def _make_runner(t_scan):
    """Compile once, return a callable in_maps -> list[dict] that only
    executes (PJRT executable cached across kernel() calls)."""
    import jax
    import jax.numpy as jnp
    import numpy as np
    from jax.sharding import Mesh, PartitionSpec
    from jax.experimental.shard_map import shard_map
    from concourse import bass2jax, mybir
    from concourse.bass2jax import _bass_exec_p, install_neuronx_cc_hook

    install_neuronx_cc_hook()
    nc = _get_nc(t_scan)
    assert nc.dbg_addr is None
    n_cores = N_CORES
    partition_name = (nc.partition_id_tensor.name
                      if nc.partition_id_tensor else None)
    in_names, out_names, out_avals, zero_shapes = [], [], [], []
    for alloc in nc.m.functions[0].allocations:
        if not isinstance(alloc, mybir.MemoryLocationSet):
            continue
        name = alloc.memorylocations[0].name
        if alloc.kind == "ExternalInput":
            if name != partition_name:
                in_names.append(name)
        elif alloc.kind == "ExternalOutput":
            shape = tuple(alloc.tensor_shape)
            npdt = mybir.dt.np(alloc.dtype)
            out_avals.append(jax.core.ShapedArray(shape, npdt))
            out_names.append(name)
            zero_shapes.append((shape, npdt))
    n_params = len(in_names)
    n_outs = len(out_names)
    all_in = in_names + out_names
    if partition_name is not None:
        all_in = all_in + [partition_name]

    def _body(*args):
        operands = list(args)
        if partition_name is not None:
            operands.append(bass2jax.partition_id_tensor())
        outs = _bass_exec_p.bind(
            *operands,
            out_avals=tuple(out_avals),
            in_names=tuple(all_in),
            out_names=tuple(out_names),
            lowering_input_output_aliases=(),
            sim_require_finite=True,
            sim_require_nnan=True,
            nc=nc,
        )
        return tuple(outs)

    devices = jax.devices()[:n_cores]
    mesh = Mesh(np.asarray(devices), ("core",))
    donate = tuple(range(n_params, n_params + n_outs))
    sharded = jax.jit(
        shard_map(_body, mesh=mesh,
                  in_specs=(PartitionSpec("core"),) * (n_params + n_outs),
                  out_specs=(PartitionSpec("core"),) * n_outs,
                  check_rep=False),
        donate_argnums=donate, keep_unused=True,
    )

    def run(in_maps):
        concat_in = [
            np.concatenate([np.asarray(m[name]) for m in in_maps], axis=0)
            for name in in_names
        ]
        concat_zeros = [
            jnp.zeros((n_cores * s[0], *s[1:]), dt) for s, dt in zero_shapes
        ]
        out_arrs = sharded(*concat_in, *concat_zeros)
        return [
            {name: np.asarray(out_arrs[i]).reshape(
                n_cores, *out_avals[i].shape)[c]
             for i, name in enumerate(out_names)}
            for c in range(n_cores)
        ]

    run.in_names = in_names
    run.out_names = out_names
    run.zero_shapes = zero_shapes
    run.sharded = sharded
    run.n_cores = n_cores
    return run


def _get_runner(t_scan):
    if t_scan not in _RUNNERS:
        _RUNNERS[t_scan] = _make_runner(t_scan)
    return _RUNNERS[t_scan]


_GATE_PERM = None


def _gate_perm():
    global _GATE_PERM
    if _GATE_PERM is None:
        # reference gate row order i,f,g,o -> kernel order i,f,o,g
        _GATE_PERM = np.concatenate([
            np.arange(0, H), np.arange(H, 2 * H),
            np.arange(3 * H, 4 * H), np.arange(2 * H, 3 * H)])
    return _GATE_PERM


def make_in_maps(x, W_ih_f, W_hh_f, b_f, W_ih_b, W_hh_b, b_b):
    """Host-side input prep: one map per core (0=forward, 1=backward)."""
    x = np.asarray(x, dtype=np.float32)
    perm = _gate_perm()
    in_maps = []
    for d, (wih, whh, bb) in enumerate(
            [(W_ih_f, W_hh_f, b_f), (W_ih_b, W_hh_b, b_b)]):
        wihp = np.asarray(wih, np.float32)[perm]
        whhp = np.asarray(whh, np.float32)[perm]
        bp = np.asarray(bb, np.float32)[perm]
        xd = x if d == 0 else x[:, ::-1]
        xt = np.ascontiguousarray(
            xd.transpose(2, 1, 0).reshape(I, TB)).astype(np.float16)
        in_maps.append({
            "xT": xt,
            "wihT": np.ascontiguousarray(wihp.T).astype(np.float16),
            "whhT": np.ascontiguousarray(whhp.T).astype(np.float16),
            "bsb": np.ascontiguousarray(bp.reshape(M, 128).T),
        })
    return in_maps


def unscramble(results, t_scan):
    """results: list of per-core out dicts -> full [32, t_scan, 1024]."""
    n_win = t_scan // WIN
    halves = []
    for d in range(2):
        raw = np.asarray(results[d]["out_raw"])  # [n_win, 128, WIN*KH*B]
        h = raw.reshape(n_win, 128, WIN, KH, B)
        h = np.ascontiguousarray(h.transpose(4, 0, 2, 3, 1))
        h = h.reshape(B, n_win * WIN, H)[:, :t_scan]
        if d == 1:
            h = h[:, ::-1]
        halves.append(h)
    return np.concatenate(halves, axis=2).astype(np.float32)


def kernel(x, W_ih_f, W_hh_f, b_f, W_ih_b, W_hh_b, b_b, _t_scan=T_SCAN):
    in_maps = make_in_maps(x, W_ih_f, W_hh_f, b_f, W_ih_b, W_hh_b, b_b)
    try:
        runner = _get_runner(_t_scan)
        results = runner(in_maps)
    except Exception:
        from concourse.bass_utils import run_bass_kernel_spmd
        res = run_bass_kernel_spmd(_get_nc(_t_scan), in_maps,
                                   list(range(N_CORES)))
        results = res.results
    return unscramble(results, _t_scan)


# revision 6
# speedup vs baseline: 462.4596x; 15.5173x over previous
"""Bidirectional LSTM on trn2 NeuronCores.

Sharding: 2 cores, one per direction, full batch B=32 per core. The
backward core receives time-reversed x and its output is re-reversed on
the host. The scan is fully core-local (the recurrence never crosses the
wire), and using 2 cores instead of 8 minimizes total device-seconds:
the scan cost is dominated by W_hh stationary-weight ingestion into the
PE array (64 LDWEIGHTS x 128x128 fp16 per step), which is independent of
the per-core batch size, so batch-splitting across more cores multiplies
device time without reducing latency.

Per-core plan (B=32, T=512, I=256, H=512, G=4H=2048):
  1. Host pre-transposes/casts weights and x to fp16 (lhsT / moving
     layouts, t-major x). Gate blocks permuted to i,f,o,g so sigmoid
     covers one contiguous 96-col span per chunk.
  2. Phase C: xp = x @ W_ih.T + b for all T, written to a DRAM buffer in
     16-step window layout (doesn't fit SBUF at B=32), N=512 moving
     columns per matmul so LDWEIGHTS is fully amortized.
  3. 512-step scan, chunk-pipelined: gates are computed per H-chunk k
     (16 matmuls -> PSUM tile [128, 4x32]), and each chunk's
     DVE/ACT tail (add xp, sigmoid/tanh, cell update) runs while the PE
     streams the next chunk's weights. h chunks are written straight
     into the fp16 output window tile, which doubles as next step's
     moving operand, so the serial tail at a step boundary is one chunk
     deep instead of a full step.
  4. xp windows stream DRAM->SBUF double-buffered; output windows
     (16 steps) DMA out as they complete; host unscrambles + upcasts.

The compiled PJRT executable is cached at module level: repeat kernel()
calls only transfer fresh inputs and execute.
"""

import numpy as np

B_FULL, T, I, H = 32, 512, 256, 512
G = 4 * H
N_CORES = 2
B = B_FULL                # per-core batch (one direction per core)
KH = H // 128             # 4 contraction chunks for W_hh
KI = I // 128             # 2 contraction chunks for W_ih
M = G // 128              # 16 gate-row chunks (4 per gate)
WIN = 16                  # scan steps per xp/output window
TB = T * B                # 16384 moving columns, t-major
WCOL = M * WIN * B        # 8192 xp columns per window
T_SCAN = T

_BUILT = {}


def _install_tile_patch():
    """This container's walrus accepts only ONE sync-wait per CTRL-class
    instruction (Drain/NoOp). Tile's kernel-tail drain aggregates one wait
    per semaphore lane onto a single Drain -> split them one per drain."""
    import bass_rust
    import concourse.tile as tile

    if getattr(tile.TileContext, "_drain_split_patched", False):
        return

    def _patched_dab(self, tick_clock, wait_clock):
        from concourse.tile import ScopedClock

        nc = self.nc
        drain_inst = nc.sync.drain()
        wait_clock.add_sem_waits(
            drain_inst.ins, ScopedClock({None: tick_clock.global_clock})
        )
        si = drain_inst.ins.sync_info
        waits = list(si.on_wait) if si is not None else []
        if len(waits) > 1:
            si.on_wait = waits[:1]
            for w in waits[1:]:
                d2 = nc.sync.drain()
                si2 = d2.ins.sync_info
                if si2 is None:
                    d2.ins.sync_info = bass_rust.SyncInfo(on_wait=[w], on_update=[])
                else:
                    si2.on_wait = list(si2.on_wait) + [w]
        nc.all_engine_barrier()
        assert self.sems is not None
        popped = nc._tile_sem_poison_stack.pop()
        assert popped is self._sem_poison
        nc.clear_and_free_semaphores(list(self.sems.allocated().values()))
        nc.all_engine_barrier()

    tile.TileContext._drain_and_barrier = _patched_dab
    tile.TileContext._drain_split_patched = True

    # This walrus build accepts at most ONE sync-wait per instruction (any
    # opcode). Split every multi-wait instruction at BIR-JSON level into
    # single-wait NoOps followed by the real instruction with one wait.
    import json
    import concourse.bass as bass

    if getattr(bass.Bass, "_json_wait_split_patched", False):
        return
    _orig_tjb = bass.Bass.to_json_bytes

    def _split_json(self):
        raw = _orig_tjb(self)
        m = json.loads(raw)
        ctr = 0
        changed = False
        for fn in m.get("functions", []):
            for bb in fn.get("blocks", []):
                out = []
                for inst in bb.get("instructions", []):
                    si = inst.get("sync_info")
                    waits = (si or {}).get("on_wait") or []
                    if len(waits) > 1:
                        changed = True
                        for w in waits[:-1]:
                            ctr += 1
                            nop = {
                                "engine": inst["engine"],
                                "ins": [],
                                "outs": [],
                                "name": f"WSPLIT-{ctr}",
                                "opcode": "NoOp",
                                "sync_info": {"on_update": [], "on_wait": [w]},
                            }
                            if "debug" in inst:
                                nop["debug"] = inst["debug"]
                            out.append(nop)
                        si["on_wait"] = [waits[-1]]
                    out.append(inst)
                bb["instructions"] = out
        if not changed:
            return raw
        return json.dumps(m).encode()

    bass.Bass.to_json_bytes = _split_json
    bass.Bass._json_wait_split_patched = True


def _build(t_scan):
    import concourse.bass as bass
    import concourse.tile as tile
    from concourse import mybir
    from contextlib import ExitStack

    _install_tile_patch()
    f32 = mybir.dt.float32
    f16 = mybir.dt.float16

    assert t_scan % WIN == 0
    n_win = t_scan // WIN

    nc = bass.Bass()
    # Host layouts: xT [I, T*B] f16 t-major (col t*B + b), wihT [I, G] f16,
    # whhT [H, G] f16 (G rows permuted to gate order i,f,o,g; g-gate rows
    # pre-scaled by 2 so tanh(x) = 2*sigmoid(2x)-1 folds into the single
    # sigmoid pass), bsb [128, M], ident = eye(128) f16.
    xt_d = nc.dram_tensor("xT", [I, TB], f16, kind="ExternalInput")
    wiht_d = nc.dram_tensor("wihT", [I, G], f16, kind="ExternalInput")
    whht_d = nc.dram_tensor("whhT", [H, G], f16, kind="ExternalInput")
    bsb_d = nc.dram_tensor("bsb", [128, M], f32, kind="ExternalInput")
    id_d = nc.dram_tensor("ident", [128, 128], f16, kind="ExternalInput")
    # out[w, p, s*128 + k*32 + b] = h[b, 16w+s, 128k+p]
    out_d = nc.dram_tensor("out_raw", [n_win, 128, WIN * KH * B], f16,
                           kind="ExternalOutput")

    with tile.TileContext(nc) as tc, ExitStack() as ctx:
        sig = mybir.ActivationFunctionType.Sigmoid
        tanh = mybir.ActivationFunctionType.Tanh

        wpool = ctx.enter_context(tc.tile_pool(name="w", bufs=1))
        dpool = ctx.enter_context(tc.tile_pool(name="d", bufs=1, space="DRAM"))
        whhT = wpool.tile([128, KH * G], f16)    # tile (kk,m) at (kk*M+m)*128
        wihT = wpool.tile([128, KI * G], f16)
        xT = wpool.tile([128, KI * TB], f16)     # chunk ki at ki*TB
        b_sb = wpool.tile([128, M], f32)
        ident = wpool.tile([128, 128], f16)
        # xp DRAM buffer, window layout: col w*WCOL + m*(WIN*B) + s*B + b
        xp_dram = dpool.tile([128, n_win * WCOL], f16)
        nc.gpsimd.dma_start(b_sb[:], bsb_d[:])
        nc.gpsimd.dma_start(ident[:], id_d[:])
        for k in range(KH):
            nc.gpsimd.dma_start(whhT[:, k * G:(k + 1) * G],
                                whht_d[k * 128:(k + 1) * 128, :])
        for k in range(KI):
            nc.gpsimd.dma_start(wihT[:, k * G:(k + 1) * G],
                                wiht_d[k * 128:(k + 1) * 128, :])
            nc.gpsimd.dma_start(xT[:, k * TB:(k + 1) * TB],
                                xt_d[k * 128:(k + 1) * 128, :])

        # ---- phase C: xp = x @ W_ih.T + b -> DRAM, fp16 ----
        NXP = WIN * B  # 512 moving columns = one window of one m-chunk
        with tc.tile_pool(name="xpps", bufs=4, space="PSUM") as xpp, \
             tc.tile_pool(name="xpsb", bufs=4) as xsb:
            for w in range(n_win):
                for m in range(M):
                    ps = xpp.tile([128, NXP], f32, tag="xps")
                    for k in range(KI):
                        nc.tensor.matmul(
                            ps[:],
                            wihT[:, (k * M + m) * 128:(k * M + m + 1) * 128],
                            xT[:, k * TB + w * NXP:k * TB + (w + 1) * NXP],
                            start=(k == 0), stop=(k == KI - 1),
                        )
                    sb = xsb.tile([128, NXP], f16, tag="xsb")
                    if m % 2 == 0:
                        nc.vector.tensor_scalar_add(sb[:], ps[:], b_sb[:, m:m + 1])
                    else:
                        nc.scalar.add(sb[:], ps[:], b_sb[:, m:m + 1])
                    nc.gpsimd.dma_start(
                        xp_dram[:, w * WCOL + m * NXP:w * WCOL + (m + 1) * NXP],
                        sb[:])

        # ---- phase D: the scan ----
        # gate m-chunk = g*4 + k (g in i,f,o,g order; k = H 128-chunk)
        # h/c col layout: k*32 + b
        with tc.tile_pool(name="gp", bufs=8, space="PSUM") as gp, \
             tc.tile_pool(name="xpw", bufs=2) as xpool, \
             tc.tile_pool(name="acts", bufs=4) as ap, \
             tc.tile_pool(name="state", bufs=2) as stp, \
             tc.tile_pool(name="outb", bufs=2) as obp, \
             tc.tile_pool(name="init", bufs=1) as ip:
            h0 = ip.tile([128, KH * B], f16)
            c0 = ip.tile([128, KH * B], f32)
            nc.vector.memset(h0[:], 0.0)
            nc.vector.memset(c0[:], 0.0)

            def load_window(w):
                tl = xpool.tile([128, WCOL], f16, tag="xp")
                nc.gpsimd.dma_start(tl[:], xp_dram[:, w * WCOL:(w + 1) * WCOL])
                return tl

            xpw_cur = load_window(0)
            xpw_next = None
            h_src = h0
            c_prev = c0
            ob = None
            for t in range(t_scan):
                w, s = divmod(t, WIN)
                if s == 0:
                    if w > 0:
                        xpw_cur = xpw_next
                    if w + 1 < n_win:
                        xpw_next = load_window(w + 1)
                    ob = obp.tile([128, WIN * KH * B], f16, tag="ob")
                # xp view: [p, g, k, s, b]
                xp5 = xpw_cur.rearrange("p (g k s b) -> p g k s b",
                                        g=4, k=KH, s=WIN)
                c_t = stp.tile([128, KH * B], f32, tag="c")
                for k in range(KH):
                    ps = gp.tile([128, 4 * B], f32, tag="ps")  # i|f|o|g~
                    # preload xp into PSUM (identity matmul, 3D moving AP)
                    nc.tensor.matmul(ps[:], ident[:], xp5[:, :, k, s, :],
                                     start=True, stop=False)
                    for g in range(4):
                        m = g * KH + k
                        for kk in range(KH):
                            nc.tensor.matmul(
                                ps[:, g * B:(g + 1) * B],
                                whhT[:, (kk * M + m) * 128:(kk * M + m + 1) * 128],
                                h_src[:, kk * B:(kk + 1) * B],
                                start=False, stop=(kk == KH - 1),
                            )
                    # tail for chunk k: af = sigmoid over all 4 blocks;
                    # g~ = sigmoid(2*g_pre) (host pre-scaled), so
                    # i*g = 2*(g~ - 0.5)*i and c = 2*q + f*c_prev.
                    af = ap.tile([128, 4 * B], f32, tag="af")
                    nc.scalar.activation(af[:], ps[:], sig)
                    q = ap.tile([128, B], f32, tag="q")
                    nc.vector.scalar_tensor_tensor(
                        q[:], af[:, 3 * B:4 * B], 0.5, af[:, 0:B],
                        op0=mybir.AluOpType.subtract, op1=mybir.AluOpType.mult)
                    fc = ap.tile([128, B], f32, tag="fc")
                    nc.gpsimd.tensor_mul(fc[:], af[:, B:2 * B],
                                         c_prev[:, k * B:(k + 1) * B])
                    nc.vector.scalar_tensor_tensor(
                        c_t[:, k * B:(k + 1) * B], q[:], 2.0, fc[:],
                        op0=mybir.AluOpType.mult, op1=mybir.AluOpType.add)
                    th = ap.tile([128, B], f32, tag="th")
                    nc.scalar.activation(th[:], c_t[:, k * B:(k + 1) * B], tanh)
                    nc.vector.tensor_mul(
                        ob[:, s * KH * B + k * B:s * KH * B + (k + 1) * B],
                        af[:, 2 * B:3 * B], th[:])
                h_src = ob[:, s * KH * B:(s + 1) * KH * B]
                c_prev = c_t
                if s == WIN - 1:
                    nc.gpsimd.dma_start(out_d[w], ob[:])

    return nc


def _get_nc(t_scan):
    key = t_scan
    if key not in _BUILT:
        _BUILT[key] = _build(t_scan)
    return _BUILT[key]


_RUNNERS = {}


def _make_runner(t_scan):
    """Compile once, return a callable in_maps -> list[dict] that only
    executes (PJRT executable cached across kernel() calls)."""
    import jax
    import jax.numpy as jnp
    import numpy as np
    from jax.sharding import Mesh, PartitionSpec
    from jax.experimental.shard_map import shard_map
    from concourse import bass2jax, mybir
    from concourse.bass2jax import _bass_exec_p, install_neuronx_cc_hook

    install_neuronx_cc_hook()
    nc = _get_nc(t_scan)
    assert nc.dbg_addr is None
    n_cores = N_CORES
    partition_name = (nc.partition_id_tensor.name
                      if nc.partition_id_tensor else None)
    in_names, out_names, out_avals, zero_shapes = [], [], [], []
    for alloc in nc.m.functions[0].allocations:
        if not isinstance(alloc, mybir.MemoryLocationSet):
            continue
        name = alloc.memorylocations[0].name
        if alloc.kind == "ExternalInput":
            if name != partition_name:
                in_names.append(name)
        elif alloc.kind == "ExternalOutput":
            shape = tuple(alloc.tensor_shape)
            npdt = mybir.dt.np(alloc.dtype)
            out_avals.append(jax.core.ShapedArray(shape, npdt))
            out_names.append(name)
            zero_shapes.append((shape, npdt))
    n_params = len(in_names)
    n_outs = len(out_names)
    all_in = in_names + out_names
    if partition_name is not None:
        all_in = all_in + [partition_name]

    def _body(*args):
        operands = list(args)
        if partition_name is not None:
            operands.append(bass2jax.partition_id_tensor())
        outs = _bass_exec_p.bind(
            *operands,
            out_avals=tuple(out_avals),
            in_names=tuple(all_in),
            out_names=tuple(out_names),
            lowering_input_output_aliases=(),
            sim_require_finite=True,
            sim_require_nnan=True,
            nc=nc,
        )
        return tuple(outs)

    devices = jax.devices()[:n_cores]
    mesh = Mesh(np.asarray(devices), ("core",))
    donate = tuple(range(n_params, n_params + n_outs))
    sharded = jax.jit(
        shard_map(_body, mesh=mesh,
                  in_specs=(PartitionSpec("core"),) * (n_params + n_outs),
                  out_specs=(PartitionSpec("core"),) * n_outs,
                  check_rep=False),
        donate_argnums=donate, keep_unused=True,
    )

    def run(in_maps):
        concat_in = [
            np.concatenate([np.asarray(m[name]) for m in in_maps], axis=0)
            for name in in_names
        ]
        concat_zeros = [
            jnp.zeros((n_cores * s[0], *s[1:]), dt) for s, dt in zero_shapes
        ]
        out_arrs = sharded(*concat_in, *concat_zeros)
        return [
            {name: np.asarray(out_arrs[i]).reshape(
                n_cores, *out_avals[i].shape)[c]
             for i, name in enumerate(out_names)}
            for c in range(n_cores)
        ]

    run.in_names = in_names
    run.out_names = out_names
    run.zero_shapes = zero_shapes
    run.sharded = sharded
    run.n_cores = n_cores
    return run


def _get_runner(t_scan):
    if t_scan not in _RUNNERS:
        _RUNNERS[t_scan] = _make_runner(t_scan)
    return _RUNNERS[t_scan]


_GATE_PERM = None


def _gate_perm():
    global _GATE_PERM
    if _GATE_PERM is None:
        # reference gate row order i,f,g,o -> kernel order i,f,o,g
        _GATE_PERM = np.concatenate([
            np.arange(0, H), np.arange(H, 2 * H),
            np.arange(3 * H, 4 * H), np.arange(2 * H, 3 * H)])
    return _GATE_PERM


def make_in_maps(x, W_ih_f, W_hh_f, b_f, W_ih_b, W_hh_b, b_b):
    """Host-side input prep: one map per core (0=forward, 1=backward)."""
    x = np.asarray(x, dtype=np.float32)
    perm = _gate_perm()
    in_maps = []
    ident = np.eye(128, dtype=np.float16)
    for d, (wih, whh, bb) in enumerate(
            [(W_ih_f, W_hh_f, b_f), (W_ih_b, W_hh_b, b_b)]):
        wihp = np.asarray(wih, np.float32)[perm].copy()
        whhp = np.asarray(whh, np.float32)[perm].copy()
        bp = np.asarray(bb, np.float32)[perm].copy()
        # pre-scale g-gate rows by 2: tanh(x) = 2*sigmoid(2x) - 1
        wihp[3 * H:] *= 2.0
        whhp[3 * H:] *= 2.0
        bp[3 * H:] *= 2.0
        xd = x if d == 0 else x[:, ::-1]
        xt = np.ascontiguousarray(
            xd.transpose(2, 1, 0).reshape(I, TB)).astype(np.float16)
        in_maps.append({
            "xT": xt,
            "wihT": np.ascontiguousarray(wihp.T).astype(np.float16),
            "whhT": np.ascontiguousarray(whhp.T).astype(np.float16),
            "bsb": np.ascontiguousarray(bp.reshape(M, 128).T),
            "ident": ident,
        })
    return in_maps


def unscramble(results, t_scan):
    """results: list of per-core out dicts -> full [32, t_scan, 1024]."""
    n_win = t_scan // WIN
    halves = []
    for d in range(2):
        raw = np.asarray(results[d]["out_raw"])  # [n_win, 128, WIN*KH*B]
        h = raw.reshape(n_win, 128, WIN, KH, B)
        h = np.ascontiguousarray(h.transpose(4, 0, 2, 3, 1))
        h = h.reshape(B, n_win * WIN, H)[:, :t_scan]
        if d == 1:
            h = h[:, ::-1]
        halves.append(h)
    return np.concatenate(halves, axis=2).astype(np.float32)


def kernel(x, W_ih_f, W_hh_f, b_f, W_ih_b, W_hh_b, b_b, _t_scan=T_SCAN):
    in_maps = make_in_maps(x, W_ih_f, W_hh_f, b_f, W_ih_b, W_hh_b, b_b)
    try:
        runner = _get_runner(_t_scan)
        results = runner(in_maps)
    except Exception:
        from concourse.bass_utils import run_bass_kernel_spmd
        res = run_bass_kernel_spmd(_get_nc(_t_scan), in_maps,
                                   list(range(N_CORES)))
        results = res.results
    return unscramble(results, _t_scan)
